# revision 46
# baseline (speedup 1.0000x reference)
"""Trainium2 Bass kernel for nn_BERTSyntaxRel (biaffine syntax-relation head).

Computation (per batch b, token t):
    appended = concat([root, x[b]])                      # (S+1, D)
    gathered = appended[head_id[b, t]]                   # (D,)
    head = relu(gathered @ Wh + bh)                      # (H,)
    tail = relu(x[b, t] @ Wt + bt)                       # (H,)
    out[b, t, r] = sum_{h,k} head[h] * K[h, r, k] * tail[k]

Sharding: data-parallel over batch, 4 batches per core on 8 cores.

v3 design (v2 + deep-pipeline scheduling for the timeline model):
- FF head branch token-major (lhsT = xT chunks -> hp[t,h]), tail branch
  feature-major (lhsT = Wt chunks -> tp[k,t]); ONE fused ACT relu per tile
  covers both halves of the FF psum (head bf16 rows for the DRAM gather
  table, tail bf16 columns staged in an SBUF ring for the muls).
- per-batch head tables in DRAM (bf16 rows); dma_gather(transpose=True)
  fetches 512 rows per SWDGE launch, landing TRANSPOSED as headT[h,t].
  Gathers are emitted GLEAD tiles early on the Pool stream so the PE never
  waits; B-phase lags A by LAG (> TPB) tiles so even batch-boundary
  gathers have slack behind the last table-write DMA.
- biaffine k-major: per r, U3_r[k,t] = sum_h K[h, r*H+k] * headT[h,t]
  (48 x 128-col matmuls per tile into 2-bank PSUM pairs); the elementwise
  mul with tail runs per 1024-wide pair on a route mix (dd = DVE mul from
  PSUM; ag = ACT copy to bf16 SBUF then DVE 2x half + gpsimd half; agq
  defers the DVE halves one tile to fill DVE gaps); the reduce over k runs
  on the PE as 1-col ones-matmuls (free in the cost model), deferred DEFER
  tiles and flushed at step START so their deps are long-satisfied and the
  in-order PE sequencer never head-of-line blocks.
"""

import os as _os

import numpy as np

B, S, D, H, R = 32, 1024, 768, 128, 48
NCORES = 8
BPC = B // NCORES            # batches per core (4)
TOK = BPC * S                # tokens per core (4096)
P = 128                      # partition dim / token tile
NTILES = TOK // P            # 32 token tiles per core
TPB = S // P                 # tiles per batch (8)
TBL = S + 1                  # rows per batch gather table (1025)
DC = D // P                  # 6 contraction chunks of 128
NG = 12                      # consume groups per tile (4 r's each)
RPG = R // NG                # r values per group (4)

# paired mode: 1024-wide PSUM groups; per-pair routes over NG//2 pairs:
#   dd = one DVE mul over both halves (direct from PSUM)
#   ag = one ACT copy of the pair, then DVE 2x mul half0 + gpsimd mul half1
#   aa = one ACT copy, DVE 2x muls both halves
#   gg = one ACT copy, gpsimd muls both halves
PROUTES = _os.environ.get("K_PROUTES", "dd,ag,ag,dd,ag,dd").split(",")
PSU3 = int(_os.environ.get("K_PSU3", "3"))   # psU3 bufs (2 banks each)
PSFF = int(_os.environ.get("K_PSFF", "2"))   # psFF bufs (1 bank each)
LAG = int(_os.environ.get("K_LAG", "13"))    # B-phase lag in tiles (>= TPB)
DEFER = int(_os.environ.get("K_DEFER", "1"))  # reduce deferral in tiles
AGD = int(_os.environ.get("K_AGD", "1"))     # ag DVE-half deferral in tiles
RPOS = _os.environ.get("K_RPOS", "end")      # reduce flush point: start|mid|end
GLEAD = int(_os.environ.get("K_GLEAD", "2"))  # gather lead in tiles
GLEAD0 = int(_os.environ.get("K_GLEAD0", "2"))  # first-gather lead
GPRIO = int(_os.environ.get("K_GPRIO", "0"))  # gather scheduler-priority hoist
XA = int(_os.environ.get("K_XA", "7"))       # xT prefetch depth (pairs)
WARM = int(_os.environ.get("K_WARM", "60"))  # PE p-state warmup transposes
KSBSTEP = int(_os.environ.get("K_KSBSTEP", "5"))  # step at which ksb loads
GB = 4                                        # tiles per indirect gather
XB = int(_os.environ.get("K_XB", "4"))       # tiles per xT load
OB = int(_os.environ.get("K_OB", "1"))       # tiles per output store
HB = int(_os.environ.get("K_HB", "8"))       # tiles per head-table write
OCDVE = _os.environ.get("K_OCDVE", "0") == "1"  # psO out-copy on DVE not ACT
PRODB = int(_os.environ.get("K_PRODB", "24"))  # prod ring (per tag)
CPB = int(_os.environ.get("K_CPB", "20"))     # cp ring
HBUF = int(_os.environ.get("K_HBUF", "8"))   # fused-relu staging ring
QSPREAD = _os.environ.get("K_QSPREAD", "1") == "1"  # spread DMA queues
KSPLIT = int(_os.environ.get("K_KSPLIT", "1"))  # ksb load chunks
XPACE = int(_os.environ.get("K_XPACE", "8"))  # xt pacing lead (tiles)
XHOLD = int(_os.environ.get("K_XHOLD", "10"))  # startup xt hold step
KSTEP = int(_os.environ.get("K_KSTEP", "5"))  # ksb chunk-0 emission step
AGMERGE = _os.environ.get("K_AGMERGE", "0") == "1"  # merge ag a-halves


def build_program(with_bias=False):
    """Build the Bass program (shared by all 8 cores, SPMD)."""
    from contextlib import ExitStack

    import concourse.bass as bass
    import concourse.tile as tile
    from concourse import bacc, mybir
    from concourse.masks import make_identity

    f32 = mybir.dt.float32
    bf16 = mybir.dt.bfloat16
    ts = bass.ts

    nc = bacc.Bacc(
        "TRN2",
        target_bir_lowering=False,
        debug=False,
        num_devices=NCORES,
    )

    i16 = mybir.dt.int16
    GBT = GB               # tiles per dma_gather (>512 idxs crashes the DGE)
    GN = GBT * P           # idxs per gather (512)
    IDXC = GN // 16        # idx columns per gather group (32)
    NGRP = NTILES // GBT   # gather groups per core

    xT_ap = nc.dram_tensor("xT", [D, TOK], bf16, kind="ExternalInput").ap()
    gidx_ap = nc.dram_tensor("gidx", [P, NGRP * IDXC], i16,
                             kind="ExternalInput").ap()
    ww_ap = nc.dram_tensor("WW", [P, 2 * DC * H], bf16,
                           kind="ExternalInput").ap()
    bh_ap = nc.dram_tensor("bh", [1, H], bf16, kind="ExternalInput").ap()
    bt_ap = nc.dram_tensor("bt", [1, H], f32, kind="ExternalInput").ap()
    rooth_ap = nc.dram_tensor("rooth", [BPC, H], bf16, kind="ExternalInput").ap()
    kern_ap = nc.dram_tensor("kern", [H, R * H], bf16, kind="ExternalInput").ap()
    out_ap = nc.dram_tensor("out", [TOK, R], f32, kind="ExternalOutput").ap()

    with tile.TileContext(nc) as tc, ExitStack() as ctx:
        # ---- constants / weights, resident for the whole kernel ----
        const = ctx.enter_context(tc.tile_pool(name="const", bufs=1))
        ident = const.tile([P, P], bf16)
        make_identity(nc, ident[:])
        ones_sb = const.tile([P, 1], bf16)
        nc.gpsimd.memset(ones_sb[:], 1.0)
        ones1 = const.tile([1, P], bf16)
        nc.gpsimd.memset(ones1[:], 1.0)

        # DMA queue spread: per-class queue selection (s=SP, a=ACT, p=Pool)
        def qsel(ch):
            return {"s": nc.sync, "a": nc.scalar, "p": nc.gpsimd}[ch]

        q_w = qsel(_os.environ.get("K_QW", "s"))   # startup consts
        q_k = qsel(_os.environ.get("K_QK", "s"))   # ksb chunks
        q_t = qsel(_os.environ.get("K_QT", "s"))   # head-table writes
        q_o = qsel(_os.environ.get("K_QO", "s"))   # out stores

        wwsb = const.tile([P, 2 * DC * H], bf16)  # [d-in-chunk, (c,h)|(c,k)]
        whsb = wwsb[:, : DC * H]
        wtsb = wwsb[:, DC * H :]
        q_w.dma_start(out=wwsb[:], in_=ww_ap[:])
        bh_sb = const.tile([1, H], bf16)
        bt_sb = const.tile([P, 1], f32)
        rt_sb = const.tile([BPC, H], bf16)
        if with_bias:
            q_w.dma_start(out=bh_sb[:], in_=bh_ap[:])
            q_w.dma_start(out=bt_sb[:], in_=bt_ap.rearrange("o k -> k o"))
        ksb = const.tile([H, R * H], bf16)     # 12KB/partition

        gidx_sb = const.tile([P, NGRP * IDXC], i16)

        # per-batch gather tables in DRAM: row b*TBL is the root head state
        dram = ctx.enter_context(tc.tile_pool(name="dram", bufs=1, space="DRAM"))
        head_all = dram.tile([BPC * TBL, H], bf16)
        tbl_writes = [[] for _ in range(BPC)]

        xT4 = xT_ap.rearrange("(c p) t -> p c t", p=P)  # [128, 6, TOK]

        # spin the PE while the first xT DMAs are in flight so the clock is
        # at full p-state when real work lands (ramp: 3us of continuous busy)
        if WARM > 0:
            with tc.tile_pool(name="warm", bufs=1, space="PSUM") as warmp:
                wps = warmp.tile([P, P], bf16, name="wps")
                for _ in range(WARM):
                    nc.tensor.transpose(out=wps[:], in_=ident[:],
                                        identity=ident[:])

        with (
            tc.tile_pool(name="xa", bufs=XA) as xa_pool,
            tc.tile_pool(name="hb", bufs=HBUF) as hb_pool,
            tc.tile_pool(name="gb", bufs=3) as gb_pool,
            tc.tile_pool(name="prod", bufs=PRODB) as prod_pool,
            tc.tile_pool(name="prodm", bufs=5) as prodm_pool,
            tc.tile_pool(name="cp", bufs=CPB) as cp_pool,
            tc.tile_pool(name="ob", bufs=2) as ob_pool,
            tc.tile_pool(name="psFF", bufs=PSFF, space="PSUM") as psFF,
            tc.tile_pool(name="psU3", bufs=PSU3, space="PSUM") as psU3,
        ):
            state = {"redq": [], "agq": [], "tiles": {}, "psA": {},
                     "hTb": {}, "hb": {}, "xt": {}, "relu_ins": {},
                     "gather_ins": {}, "xt_pending": []}

            def flush_ag(entry):
                if AGMERGE:
                    ti, ar, arpr, pairs = entry
                    n = len(pairs)
                    tl_ti = (
                        state["hb"][ti]
                        .rearrange("p (o k) -> p o k", o=1)
                        .to_broadcast([P, n * RPG, H])
                    )
                    nc.vector.tensor_tensor(
                        out=arpr[:].rearrange("p (r k) -> p r k", k=H),
                        in0=ar[:].rearrange("p (r k) -> p r k", k=H),
                        in1=tl_ti, op=mybir.AluOpType.mult,
                    )
                    for o, g in enumerate(pairs):
                        state["redq"].append(
                            (arpr[:, o * RPG * H : (o + 1) * RPG * H], ti, g))
                    return
                cp_h, prod_h, ti, g = entry
                tl_ti = (
                    state["hb"][ti]
                    .rearrange("p (o k) -> p o k", o=1)
                    .to_broadcast([P, RPG, H])
                )
                nc.vector.tensor_tensor(
                    out=prod_h.rearrange("p (r k) -> p r k", k=H),
                    in0=cp_h.rearrange("p (r k) -> p r k", k=H),
                    in1=tl_ti, op=mybir.AluOpType.mult,
                )
                state["redq"].append((prod_h, ti, g))

            def flush_reduce(entry):
                prod, ti, g = entry
                pso, left = state["tiles"][ti]
                for q in range(RPG):
                    nc.tensor.matmul(
                        out=pso[:, g * RPG + q : g * RPG + q + 1],
                        lhsT=prod[:, q * H : (q + 1) * H], rhs=ones_sb[:, :1],
                        start=True, stop=True,
                    )
                left -= 1
                state["tiles"][ti][1] = left
                if left == 0:
                    # tile ti fully reduced: evacuate psO and store
                    if ti % OB == 0:
                        state["ob"] = ob_pool.tile([P, OB * R], f32, name="ob")
                    ob = state["ob"]
                    if OCDVE:
                        nc.vector.tensor_copy(out=ob[:, ts(ti % OB, R)],
                                              in_=pso[:])
                    else:
                        nc.scalar.copy(out=ob[:, ts(ti % OB, R)], in_=pso[:])
                    del state["tiles"][ti]
                    if ti % OB == OB - 1:
                        q_o.dma_start(
                            out=out_ap[(ti - (OB - 1)) * P : (ti + 1) * P, :]
                            .rearrange("(j t) r -> t j r", j=OB),
                            in_=ob[:].rearrange("t (j r) -> t j r", j=OB),
                        )

            def load_xt(j):
                # xT load for tiles [XB*j, XB*(j+1)); paced by an old relu so
                # the SP queue never floods the serial DMA device ahead of
                # the table-write -> gather critical chain
                xt2 = xa_pool.tile([P, DC * XB * P], bf16, name="xt2")
                w = nc.sync.dma_start(
                    out=xt2[:].rearrange("p (c t) -> p c t", c=DC),
                    in_=xT4[:, :, j * XB * P : (j + 1) * XB * P],
                )
                pace = XB * j - XPACE
                if pace >= 0 and pace in state["relu_ins"]:
                    tile.add_dep_helper(w.ins, state["relu_ins"][pace],
                                        sync=True, reason="xt pacing")
                if j >= 3 and 0 in state["gather_ins"]:
                    # startup: keep far xt loads from cutting ahead of the
                    # first gather on the serial DMA device
                    tile.add_dep_helper(w.ins, state["gather_ins"][0],
                                        sync=True, reason="xt behind gather0")
                state["xt"][j] = xt2

            def emit_A(i):
                ps = state["pscur"][:, : 2 * H]
                state["psA"][i] = ps
                xt2 = state["xt"][i // XB]
                if i % XB == XB - 1:
                    del state["xt"][i // XB]
                off = (i % XB) * P

                def xsl(c):
                    return xt2[:, c * XB * P + off : c * XB * P + off + P]

                for c in range(DC):
                    nc.tensor.matmul(
                        out=ps[:, :H], lhsT=xsl(c), rhs=whsb[:, ts(c, H)],
                        start=(c == 0), stop=(c == DC - 1 and not with_bias),
                    )
                if with_bias:
                    nc.tensor.matmul(
                        out=ps[:, :H], lhsT=ones1[:1, :], rhs=bh_sb[:1, :],
                        start=False, stop=True,
                    )
                for c in range(DC):
                    nc.tensor.matmul(
                        out=ps[:, H:], lhsT=wtsb[:, ts(c, H)], rhs=xsl(c),
                        start=(c == 0), stop=(c == DC - 1),
                    )

            def emit_A_relu(i):
                b = i // TPB
                ps = state["psA"].pop(i)
                if i % HB == 0:
                    state["hb2"] = hb_pool.tile([P, HB * 2 * H], bf16,
                                                name="hb2")
                hb2 = state["hb2"]
                if with_bias:
                    # separate relus: tail needs a per-partition bias
                    nc.scalar.activation(
                        out=hb2[:, ts(i % HB, 2 * H)][:, :H], in_=ps[:, :H],
                        func=mybir.ActivationFunctionType.Relu,
                    )
                    r = nc.scalar.activation(
                        out=hb2[:, ts(i % HB, 2 * H)][:, H:], in_=ps[:, H:],
                        func=mybir.ActivationFunctionType.Relu,
                        bias=bt_sb[:],
                    )
                else:
                    # ONE fused relu: head rows then tail columns
                    r = nc.scalar.activation(
                        out=hb2[:, ts(i % HB, 2 * H)], in_=ps[:],
                        func=mybir.ActivationFunctionType.Relu,
                    )
                state["relu_ins"][i] = r.ins
                state["hb"][i] = hb2[:, ts(i % HB, 2 * H)][:, H:]
                if i % HB == HB - 1:
                    row0 = b * TBL + 1 + ((i % TPB) - (HB - 1)) * P
                    # head halves of each tile slot: strided [P, HB, H] view
                    w = q_t.dma_start(
                        out=head_all[row0 : row0 + HB * P, :].rearrange(
                            "(j t) h -> t j h", j=HB
                        ),
                        in_=hb2[:].rearrange("t (j s) -> t j s", j=HB)[:, :, :H],
                    )
                    tbl_writes[b].append(w.ins)

            def emit_gather(grp):
                from contextlib import nullcontext
                i0 = grp * GBT
                b = i0 // TPB
                prio = (tc.high_priority(offset=GPRIO) if GPRIO > 0
                        else nullcontext())
                with prio:
                    _emit_gather(grp, b)

            def _emit_gather(grp, b):
                hTb = gb_pool.tile([P, GN], bf16, name="hTb")
                g = nc.gpsimd.dma_gather(
                    out_ap=hTb[:].rearrange("p (o t) -> p o t", o=1),
                    in_ap=head_all[:],
                    idxs_ap=gidx_sb[:, grp * IDXC : (grp + 1) * IDXC],
                    num_idxs=GN,
                    num_idxs_reg=GN,
                    elem_size=H,
                    transpose=True,
                )
                for w_ins in tbl_writes[b]:
                    tile.add_dep_helper(g.ins, w_ins, sync=True,
                                        reason="head_all RAW")
                state["hTb"][grp] = hTb
                state["gather_ins"][grp] = g.ins

            def emit_B(i):
                if AGMERGE:
                    n_ag = PROUTES.count("ag")
                    ar = prodm_pool.tile([P, n_ag * RPG * H], bf16,
                                         tag="ar")
                    arpr = prodm_pool.tile([P, n_ag * RPG * H], bf16,
                                           tag="arp")
                    state["ar"] = (ar, arpr, [])
                headT = state["hTb"][i // GBT][:, ts(i % GBT, P)]
                if i % GBT == GBT - 1:
                    del state["hTb"][i // GBT]

                # psO region shares the current step's psFF bank
                pso = state["pscur"][:, 2 * H : 2 * H + R]
                state["tiles"][i] = [pso, NG]
                tl_b = (
                    state["hb"][i]
                    .rearrange("p (o k) -> p o k", o=1)
                    .to_broadcast([P, RPG, H])
                )
                tl_b8 = (
                    state["hb"][i]
                    .rearrange("p (o k) -> p o k", o=1)
                    .to_broadcast([P, 2 * RPG, H])
                )
                for jp in range(NG // 2):
                    u3 = psU3.tile([P, 2 * RPG * H], f32)
                    for q in range(2 * RPG):
                        nc.tensor.matmul(
                            out=u3[:, ts(q, H)],
                            lhsT=ksb[:, ts(jp * 2 * RPG + q, H)], rhs=headT,
                            start=True, stop=True,
                        )
                    pr = PROUTES[jp % len(PROUTES)]
                    if pr == "da":
                        # half0: DVE direct from PSUM; half1: ACT half-copy
                        # + deferred DVE 2x mul
                        prod = prod_pool.tile([P, 2 * RPG * H], bf16,
                                              tag="pr2")
                        nc.vector.tensor_tensor(
                            out=prod[:, : RPG * H].rearrange(
                                "p (r k) -> p r k", k=H),
                            in0=u3[:, : RPG * H].rearrange(
                                "p (r k) -> p r k", k=H),
                            in1=tl_b, op=mybir.AluOpType.mult,
                        )
                        state["redq"].append((prod[:, : RPG * H], i, 2 * jp))
                        cp = cp_pool.tile([P, RPG * H], bf16, tag="cph")
                        nc.scalar.copy(out=cp[:], in_=u3[:, RPG * H :])
                        state["agq"].append(
                            (cp[:], prod[:, RPG * H :], i, 2 * jp + 1))
                    elif pr == "dd":
                        prod = prod_pool.tile([P, 2 * RPG * H], bf16,
                                              tag="pr2")
                        nc.vector.tensor_tensor(
                            out=prod[:].rearrange("p (r k) -> p r k", k=H),
                            in0=u3[:].rearrange("p (r k) -> p r k", k=H),
                            in1=tl_b8, op=mybir.AluOpType.mult,
                        )
                        state["redq"].append((prod[:, : RPG * H], i, 2 * jp))
                        state["redq"].append((prod[:, RPG * H :], i, 2 * jp + 1))
                    elif pr == "ds":
                        # split dd: two 512-wide DVE muls
                        prod = prod_pool.tile([P, 2 * RPG * H], bf16,
                                              tag="pr2")
                        for hh in range(2):
                            sl = slice(hh * RPG * H, (hh + 1) * RPG * H)
                            nc.vector.tensor_tensor(
                                out=prod[:, sl].rearrange(
                                    "p (r k) -> p r k", k=H),
                                in0=u3[:, sl].rearrange(
                                    "p (r k) -> p r k", k=H),
                                in1=tl_b, op=mybir.AluOpType.mult,
                            )
                            state["redq"].append((prod[:, sl], i, 2 * jp + hh))
                    elif pr == "GG":
                        # gpsimd muls both halves DIRECTLY from PSUM (no ACT
                        # copy) — cost-model rate is dtype-independent
                        prod = prod_pool.tile([P, 2 * RPG * H], bf16,
                                              tag="pr2")
                        for hh in range(2):
                            sl = slice(hh * RPG * H, (hh + 1) * RPG * H)
                            nc.gpsimd.tensor_tensor(
                                out=prod[:, sl].rearrange(
                                    "p (r k) -> p r k", k=H),
                                in0=u3[:, sl].rearrange(
                                    "p (r k) -> p r k", k=H),
                                in1=tl_b, op=mybir.AluOpType.mult,
                            )
                            state["redq"].append((prod[:, sl], i, 2 * jp + hh))
                    elif AGMERGE and pr == "ag":
                        # a-half into the contiguous per-tile arena (merged
                        # DVE mul later); g-half copied + gpsimd'd as usual
                        ar, arpr, pairs = state["ar"]
                        o = len(pairs)
                        nc.scalar.copy(out=ar[:, o * RPG * H : (o + 1) * RPG * H],
                                       in_=u3[:, : RPG * H])
                        pairs.append(2 * jp)
                        cpg = cp_pool.tile([P, RPG * H], bf16, tag="cpg")
                        nc.scalar.copy(out=cpg[:], in_=u3[:, RPG * H :])
                        prodg = prodm_pool.tile([P, RPG * H], bf16,
                                                tag="prg")
                        nc.gpsimd.tensor_tensor(
                            out=prodg[:].rearrange("p (r k) -> p r k", k=H),
                            in0=cpg[:].rearrange("p (r k) -> p r k", k=H),
                            in1=tl_b, op=mybir.AluOpType.mult,
                        )
                        state["redq"].append((prodg[:], i, 2 * jp + 1))
                    else:
                        cp = cp_pool.tile([P, 2 * RPG * H], bf16, tag="cp2")
                        nc.scalar.copy(out=cp[:], in_=u3[:])
                        prod = prod_pool.tile([P, 2 * RPG * H], bf16,
                                              tag="pr2")
                        prod0 = prod[:, : RPG * H]
                        prod1 = prod[:, RPG * H :]
                        # ag: gpsimd muls half1 now, DVE half0 deferred;
                        # gg: both halves gpsimd now; aa: both halves DVE
                        # deferred
                        if pr in ("ag", "gg"):
                            nc.gpsimd.tensor_tensor(
                                out=prod1[:].rearrange(
                                    "p (r k) -> p r k", k=H),
                                in0=cp[:, RPG * H :].rearrange(
                                    "p (r k) -> p r k", k=H),
                                in1=tl_b, op=mybir.AluOpType.mult,
                            )
                            state["redq"].append((prod1[:], i, 2 * jp + 1))
                        if pr == "gg":
                            nc.gpsimd.tensor_tensor(
                                out=prod0[:].rearrange(
                                    "p (r k) -> p r k", k=H),
                                in0=cp[:, : RPG * H].rearrange(
                                    "p (r k) -> p r k", k=H),
                                in1=tl_b, op=mybir.AluOpType.mult,
                            )
                            state["redq"].append((prod0[:], i, 2 * jp))
                        elif pr == "ag":
                            state["agq"].append(
                                (cp[:, : RPG * H], prod0[:], i, 2 * jp))
                        elif pr == "aa":
                            state["agq"].append(
                                (cp[:, : RPG * H], prod0[:], i, 2 * jp))
                            state["agq"].append(
                                (cp[:, RPG * H :], prod1[:], i, 2 * jp + 1))

            NSTEP = NTILES + LAG + DEFER + 1
            for step in range(NSTEP):
                ib = step - LAG  # B-tile index this step

                def flush_deferred():
                    while state["redq"] and state["redq"][0][1] <= ib - DEFER:
                        flush_reduce(state["redq"].pop(0))

                # 1) deferred DVE ag-halves for old tiles
                agq_ti = (lambda e: e[0]) if AGMERGE else (lambda e: e[2])
                while state["agq"] and agq_ti(state["agq"][0]) <= ib - AGD:
                    flush_ag(state["agq"].pop(0))
                # 2) deferred PE reduces for older tiles (deps long done)
                if RPOS == "start":
                    flush_deferred()
                # xT loads: groups 0,1 at step 0; group 2 just-in-time;
                # groups 3+ held until after the first gather is emitted
                if step == 0:
                    load_xt(0)
                    if NTILES // XB > 1:
                        load_xt(1)
                    q_w.dma_start(out=gidx_sb[:], in_=gidx_ap[:])
                elif step % XB == 0 and step // XB + 1 < NTILES // XB:
                    j = step // XB + 1
                    if j == 2 or step >= XHOLD:
                        load_xt(j)
                    else:
                        state["xt_pending"].append(j)
                if step == XHOLD:
                    for j in state["xt_pending"]:
                        load_xt(j)
                    state["xt_pending"] = []
                # 3) A phase
                if step < NTILES or ib < NTILES:
                    state["pscur"] = psFF.tile([P, 2 * H + 64], f32,
                                               name="pscur")
                if step < NTILES:
                    emit_A(step)
                if step == 1:
                    q_w.dma_start(out=rt_sb[:], in_=rooth_ap[:])
                    w = q_w.dma_start(
                        out=head_all[:, :].rearrange(
                            "(b r) h -> b r h", r=TBL)[:, 0, :],
                        in_=rt_sb[:, :],
                    )
                    for b in range(BPC):
                        tbl_writes[b].append(w.ins)
                kcw = R * H // KSPLIT
                if step == KSTEP:
                    # first ksb chunk early, in the relu-chain DMA idle gap
                    q_k.dma_start(out=ksb[:, :kcw], in_=kern_ap[:, :kcw])
                elif TPB + 2 <= step < TPB + 1 + KSPLIT:
                    # remaining chunks held behind the first gather so they
                    # don't cut ahead on the serial DMA device
                    kc = step - TPB - 1
                    w = q_k.dma_start(out=ksb[:, kc * kcw : (kc + 1) * kcw],
                                      in_=kern_ap[:, kc * kcw : (kc + 1) * kcw])
                    if 0 in state["gather_ins"]:
                        tile.add_dep_helper(w.ins, state["gather_ins"][0],
                                            sync=True,
                                            reason="ksb behind gather0")
                # 4) gather ahead of its consumer tiles — but never before
                # its batch's table-write DMAs have been emitted (real RAW)
                for g in range(NGRP):
                    lead = GLEAD0 if g == 0 else GLEAD
                    want = max(LAG + g * GBT - lead,
                               TPB * ((g * GBT) // TPB) + TPB)
                    if step == want:
                        emit_gather(g)
                if RPOS == "mid":
                    flush_deferred()
                # 5) B phase
                if 0 <= ib < NTILES:
                    emit_B(ib)
                if AGMERGE and 0 <= ib < NTILES and state["ar"][2]:
                    ar, arpr, pairs = state["ar"]
                    state["agq"].append((ib, ar, arpr, pairs))
                # 6) fused relu after B's copies (consumers are LAG away)
                if step < NTILES:
                    emit_A_relu(step)
                if RPOS == "end":
                    flush_deferred()
            while state["agq"]:
                flush_ag(state["agq"].pop(0))
            while state["redq"]:
                flush_reduce(state["redq"].pop(0))

    nc.compile()
    return nc


def prep_inputs(x, head_id, root, Wh, bh, Wt, bt, kernel):
    """Host-side prep: shard over batch, transpose+cast x, gather indices."""
    import ml_dtypes

    bf16 = ml_dtypes.bfloat16

    x = np.asarray(x, dtype=np.float32)
    head_id = np.asarray(head_id)
    root = np.asarray(root, dtype=np.float32)
    Wh = np.asarray(Wh, dtype=np.float32)
    bh = np.asarray(bh, dtype=np.float32)
    Wt = np.asarray(Wt, dtype=np.float32)
    bt = np.asarray(bt, dtype=np.float32)
    kernel = np.asarray(kernel, dtype=np.float32)

    rooth = np.tile(
        np.maximum(root @ Wh + bh, 0.0).astype(bf16).reshape(1, H), (BPC, 1)
    )
    # weight chunks: [d-in-chunk, (c, h)] so chunk c is a [128, 128] free slice
    whc = np.ascontiguousarray(
        Wh.reshape(DC, P, H).transpose(1, 0, 2).reshape(P, DC * H)
    ).astype(bf16)
    wtc = np.ascontiguousarray(
        Wt.reshape(DC, P, H).transpose(1, 0, 2).reshape(P, DC * H)
    ).astype(bf16)
    shared = {
        "WW": np.ascontiguousarray(np.concatenate([whc, wtc], axis=1)),
        "bh": bh.reshape(1, H).astype(bf16),
        "bt": bt.reshape(1, H).astype(np.float32),
        "rooth": rooth,
        "kern": kernel.astype(bf16),
    }
    GBT, GN = GB, GB * P
    IDXC = GN // 16
    NGRP = NTILES // GBT
    in_maps = []
    for c in range(NCORES):
        bs = slice(c * BPC, (c + 1) * BPC)
        hid = head_id[bs].astype(np.int64)           # (BPC, S)
        boff = (np.arange(BPC, dtype=np.int64) * TBL)[:, None]
        gidx = (hid + boff).astype(np.int16).reshape(TOK)
        # dma_gather idx layout: idx i of group g at [i % 16, g*IDXC + i // 16],
        # replicated into all eight 16-partition stripes (one per Q7 core)
        gidx_w = np.zeros((P, NGRP * IDXC), np.int16)
        for g in range(NGRP):
            blk = gidx[g * GN : (g + 1) * GN].reshape(IDXC, 16).T
            gidx_w[:, g * IDXC : (g + 1) * IDXC] = np.tile(blk, (8, 1))
        m = dict(shared)
        m["xT"] = np.ascontiguousarray(
            x[bs].reshape(TOK, D).T
        ).astype(bf16)
        m["gidx"] = gidx_w
        in_maps.append(m)
    return in_maps


_NC_CACHE = {}


def _get_program(with_bias=False):
    key = ("nc", with_bias)
    if key not in _NC_CACHE:
        _NC_CACHE[key] = build_program(with_bias=with_bias)
    return _NC_CACHE[key]


def kernel(x, head_id, root, Wh, bh, Wt, bt, kernel):
    import time

    from concourse import bass_utils

    in_maps = prep_inputs(x, head_id, root, Wh, bh, Wt, bt, kernel)
    with_bias = bool(np.any(np.asarray(bh)) or np.any(np.asarray(bt)))
    nc = _get_program(with_bias=with_bias)
    res = None
    for attempt in range(6):
        try:
            res = bass_utils.run_bass_kernel_spmd(
                nc, in_maps, core_ids=list(range(NCORES))
            )
            break
        except Exception:
            # the first execution after a fresh NEFF compile occasionally
            # fails transiently; the device recovers after a short wait
            if attempt == 5:
                raise
            time.sleep(5.0 + 10.0 * attempt)
    outs = [res.results[c]["out"].reshape(BPC, S, R) for c in range(NCORES)]
    return np.concatenate(outs, axis=0)
